# revision 11
# baseline (speedup 1.0000x reference)
"""Trainium2 Bass kernel for nn_DepthwiseSeparableFusedConv2d (v3).

Self-contained: takes FULL inputs (x [32,256,56,56] + weights), returns FULL
output [32,256,56,56].  Data-parallel over batch across 8 NeuronCores with 4
batches/core; channels on partitions in 2 groups of 128.

v3 design (v1 baseline ~968us, v2 ~472us but rel-err 1.87e-2):
 - EXACT integer depthwise conv: qdw*s_x = (s_w*s_x)*q + (mn_w*s_x)*1 with
   q the integer quant codes.  conv(k, q) runs in fp16 (ints <=255 exact,
   products exact in fp32 PSUM, fast weight load); the mn_w term is a 3x3
   box-sum = 3 vertical ident-matmul taps over a horizontally pre-summed
   fp16 tile kh (ints <=765 exact in fp16), computed on gpsimd.  The two
   PSUM groups are merged by a vector scalar_tensor_tensor which also
   harvests the per-(c,b) h1 sums via accum_out.  This removes the fp32r
   ~12-bit weight rounding that cost rel-err in v2.
 - pointwise conv as split-bf16 (hi+lo) 2-pass accumulation: k2 exact in
   bf16, weight splitting gives ~fp32 precision at bf16 speed.
 - x loaded contiguously straight into the ring ([p,3136] 12.5KB lines),
   quantized in place, k codes written to compact fp16 tiles (no padding;
   column-clipped taps + zero pads with analytic strip/corner corrections)
 - no DRAM spill of h3: 8-buffer SBUF ring (x raw -> h1 -> h3)
 - 3 collectives instead of 6 (sums ride with min/max; BN means from raw
   h sums); gpsimd partition_all_reduce for cross-channel stats
 - min/max stat reduces in phases A and F use a gpsimd tensor-tensor
   half-splitting prepass to take load off the vector engine
 - 2 warmup collectives absorb cc-stream init under the input DMA
"""

import math
import numpy as np

# ---------------------------------------------------------------- constants
P = 128
G = 2                 # channel groups (256 = 2*128)
B_FULL = 32
BL = 4                # batches per core
NCORES = 8
HH = 56
IMG = HH * HH         # 3136
MAGIC = 12582912.0    # 1.5 * 2**23  (fp32 round-to-nearest-even trick)
QMAX = 255.0
N_TOT = B_FULL * IMG  # 100352
NCHUNKS = 16
EPS = 1e-5
_N_CHUNK_EL = B_FULL * IMG // NCHUNKS
SCALE_FIX = float((0.5 * 0.35) * (1 + (math.pi * math.log(4)) ** 0.5)
                  / ((2 * math.log(_N_CHUNK_EL)) ** 0.5))

_PROGRAM_CACHE = {}


def _host_quant_codes(w):
    w = np.asarray(w, np.float32)
    mn = w.min()
    mx = w.max()
    scale = np.maximum(((mx - mn) / np.float32(QMAX)).astype(np.float32),
                       np.float32(1e-8))
    t = np.clip((w - mn) / scale, np.float32(0.0),
                np.float32(QMAX)).astype(np.float32)
    q = np.round(t).astype(np.float32)
    return q, (q * scale + mn).astype(np.float32), float(scale), float(mn)


def _host_quant(w):
    return _host_quant_codes(w)[1]


def build_program(limit=7):  # limit unused
    import concourse.bacc as bacc
    import concourse.bass_isa as bass_isa
    import concourse.mybir as mybir
    import concourse.tile as tile

    f32 = mybir.dt.float32
    bf16 = mybir.dt.bfloat16
    fp16 = mybir.dt.float16
    AL = mybir.AluOpType
    AF = mybir.ActivationFunctionType
    AX = mybir.AxisListType
    RED = bass_isa.ReduceOp

    nc = bacc.Bacc('TRN2', target_bir_lowering=False, debug=False,
                   num_devices=NCORES)

    # ------------------------------------------------ external tensors
    x_in = nc.dram_tensor('x', [BL, 256, HH, HH], f32, kind='ExternalInput')
    ident_in = nc.dram_tensor('ident', [P, P], f32, kind='ExternalInput')
    qint_in = nc.dram_tensor('qint', [G, P, 9], f32, kind='ExternalInput')
    dwsc_in = nc.dram_tensor('dwsc', [2], f32, kind='ExternalInput')
    wsum_in = nc.dram_tensor('wsum', [G, P], f32, kind='ExternalInput')
    strips_in = nc.dram_tensor('strips', [G, P, 4], f32,
                               kind='ExternalInput')
    corners_in = nc.dram_tensor('corners', [G, P, 4], f32,
                                kind='ExternalInput')
    ks_in = nc.dram_tensor('ksum', [G, P], f32, kind='ExternalInput')
    qdb_in = nc.dram_tensor('qdb', [G, P], f32, kind='ExternalInput')
    qbn1w_in = nc.dram_tensor('qbn1w', [G, P], f32, kind='ExternalInput')
    bn1b_in = nc.dram_tensor('bn1b', [G, P], f32, kind='ExternalInput')
    qbn2w_in = nc.dram_tensor('qbn2w', [G, P], f32, kind='ExternalInput')
    bn2b_in = nc.dram_tensor('bn2b', [G, P], f32, kind='ExternalInput')
    pwsum_in = nc.dram_tensor('pwsum', [G, P], f32, kind='ExternalInput')
    # pwT[kg, cin(128), (coutg, cout)] : lhsT layout, already transposed
    pwT_in = nc.dram_tensor('pwT', [G, P, 256], f32, kind='ExternalInput')
    out_d = nc.dram_tensor('out', [BL, 256, HH, HH], f32,
                           kind='ExternalOutput')

    rg = [list(range(NCORES))]
    tiles = [(g, b) for b in range(BL) for g in range(G)]  # b-major

    with tile.TileContext(nc) as tc:
        with (
            tc.tile_pool(name='perm', bufs=1) as perm,
            tc.tile_pool(name='big', bufs=8) as big,
            tc.tile_pool(name='kb', bufs=2) as kbp,
            tc.tile_pool(name='kv', bufs=2) as kvp,
            tc.tile_pool(name='k2', bufs=4) as k2p,
            tc.tile_pool(name='scr', bufs=2) as scr,
            tc.tile_pool(name='dram', bufs=1, space='DRAM') as dpool,
            tc.tile_pool(name='psA', bufs=1, space='PSUM') as pspA,
            tc.tile_pool(name='psB', bufs=1, space='PSUM') as pspB,
        ):
            # ------------------------------------------------ warmup AGs
            warm = perm.tile([1, 2], f32, name='warm')
            nc.vector.memset(warm[:], 0.0)
            for wi in range(2):
                agw_in = dpool.tile([2], f32, name=f'agw_in{wi}')
                agw_out = dpool.tile([NCORES * 2], f32, name=f'agw_out{wi}')
                nc.sync.dma_start(agw_in[None, :], warm[:])
                nc.gpsimd.collective_compute(
                    'AllGather', AL.bypass, replica_groups=rg,
                    ins=[agw_in[:].opt()], outs=[agw_out[:].opt()])

            # ------------------------------------------------ constants
            ident = perm.tile([P, P], f32, name='identsb')
            nc.sync.dma_start(ident[:], ident_in[:])
            qint = perm.tile([P, G * 9], f32, name='qintsb')
            nc.sync.dma_start(
                qint.rearrange('c (g t) -> c g t', g=G),
                qint_in.rearrange('g c t -> c g t'))
            dwsc = perm.tile([1, 2], f32, name='dwsc1')
            nc.sync.dma_start(dwsc[:], dwsc_in[None, :])
            dwscb = perm.tile([P, 2], f32, name='dwscb')
            nc.gpsimd.partition_broadcast(dwscb[:], dwsc[:])

            def load_gp(t_in, nm):
                t = perm.tile([P, G], f32, name=nm)
                nc.sync.dma_start(t[:], t_in.rearrange('g c -> c g'))
                return t
            wsum_t = load_gp(wsum_in, 'wsumsb')
            ks_t = load_gp(ks_in, 'kssb')
            qdb_t = load_gp(qdb_in, 'qdbsb')
            qbn1w_t = load_gp(qbn1w_in, 'qbn1wsb')
            bn1b_t = load_gp(bn1b_in, 'bn1bsb')
            qbn2w_t = load_gp(qbn2w_in, 'qbn2wsb')
            bn2b_t = load_gp(bn2b_in, 'bn2bsb')
            pwsum_t = load_gp(pwsum_in, 'pwsumsb')
            strips = perm.tile([P, G, 4], f32, name='stripssb')
            nc.sync.dma_start(strips[:], strips_in.rearrange('g c s -> c g s'))
            corners = perm.tile([P, G, 4], f32, name='cornerssb')
            nc.sync.dma_start(corners[:],
                              corners_in.rearrange('g c s -> c g s'))
            pwT = perm.tile([P, G * 256], f32, name='pwTsb')
            nc.sync.dma_start(pwT.rearrange('c (g m) -> c g m', g=G),
                              pwT_in[:].rearrange('g c m -> c g m'))

            # integer diag weights (fp16, exact) + fp16 identity
            diagq = perm.tile([P, G * 9 * P], fp16, name='diagq')
            for g in range(G):
                for t in range(9):
                    i = g * 9 + t
                    nc.vector.tensor_scalar(
                        diagq[:, i * P:(i + 1) * P], ident[:],
                        qint[:, i:i + 1], None, op0=AL.mult)
            identh = perm.tile([P, P], fp16, name='identh')
            nc.vector.tensor_scalar(identh[:], ident[:], 1.0, None,
                                    op0=AL.mult)

            # ------------------------------------------------ stat tiles
            # layout per group: [P, 12] = min(b0..3), max(b0..3), sum(b0..3)
            xst = [perm.tile([P, 2 * BL], f32, name=f'xst{g}')
                   for g in range(G)]
            h1st = [perm.tile([P, 3 * BL], f32, name=f'h1st{g}')
                    for g in range(G)]
            h3st = [perm.tile([P, 3 * BL], f32, name=f'h3st{g}')
                    for g in range(G)]
            Ag = [perm.tile([P, 3, NCORES, BL], f32, name=f'Ag{g}')
                  for g in range(G)]
            cstat = [perm.tile([P, 2, NCHUNKS], f32, name=f'cstat{g}')
                     for g in range(G)]
            acc2 = [perm.tile([P, 2], f32, name=f'acc2_{i}')
                    for i in range(len(tiles))]

            def pt(nm, w=1):
                return perm.tile([P, w], f32, name=nm)

            qmax_bc = pt('qmax_bc')
            nc.vector.memset(qmax_bc[:], QMAX)

            # helper: [P,1] per-sample quant params from neg-min/max planes
            def qparams(negs, maxs, F, count, tag):
                pn = []
                px = []
                for g in range(G):
                    a = perm.tile([P, F], f32, name=f'pn_{tag}{g}')
                    nc.gpsimd.partition_all_reduce(a[:], negs[g], channels=P,
                                                   reduce_op=RED.max)
                    pn.append(a)
                    b = perm.tile([P, F], f32, name=f'px_{tag}{g}')
                    nc.gpsimd.partition_all_reduce(b[:], maxs[g], channels=P,
                                                   reduce_op=RED.max)
                    px.append(b)
                nm = perm.tile([P, F], f32, name=f'nm_{tag}')
                nc.vector.tensor_tensor(nm[:], pn[0][:], pn[1][:], op=AL.max)
                mx = perm.tile([P, F], f32, name=f'mxp_{tag}')
                nc.vector.tensor_tensor(mx[:], px[0][:], px[1][:], op=AL.max)
                nsum = pt(f'nsum_{tag}')
                nc.vector.tensor_reduce(nsum[:], nm[:], axis=AX.X, op=AL.add)
                xsum = pt(f'xsum_{tag}')
                nc.vector.tensor_reduce(xsum[:], mx[:], axis=AX.X, op=AL.add)
                mn = pt(f'mn_{tag}')
                nc.vector.tensor_scalar(mn[:], nsum[:], -1.0 / count, None,
                                        op0=AL.mult)
                mxm = pt(f'mx_{tag}')
                nc.vector.tensor_scalar(mxm[:], xsum[:], 1.0 / count, None,
                                        op0=AL.mult)
                return _qp_from_mnmx(mn, mxm, tag)

            def _qp_from_mnmx(mn, mxm, tag):
                d = pt(f'd_{tag}')
                nc.vector.tensor_sub(d[:], mxm[:], mn[:])
                s = pt(f's_{tag}')
                nc.vector.tensor_scalar(s[:], d[:], 1.0 / QMAX, 1e-8,
                                        op0=AL.mult, op1=AL.max)
                inv_s = pt(f'invs_{tag}')
                nc.vector.reciprocal(inv_s[:], s[:])
                negmn = pt(f'negmn_{tag}')
                nc.vector.tensor_scalar(negmn[:], mn[:], -1.0, None,
                                        op0=AL.mult)
                bias = pt(f'bias_{tag}')
                nc.vector.tensor_mul(bias[:], negmn[:], inv_s[:])
                return {'mn': mn, 'mx': mxm, 's': s, 'inv_s': inv_s,
                        'negmn': negmn, 'bias': bias}

            # =========================================================
            # Phase A: load x (contiguous) straight into ring tiles,
            # per-(c,b) min/max (gpsimd half-split + V reduce)
            # =========================================================
            xr = {}
            for ti, (g, b) in enumerate(tiles):
                st = big.tile([P, IMG], f32, name=f'xr{g}_{b}', tag='ring')
                xr[(g, b)] = st
                nc.sync.dma_start(
                    st[:], x_in[b, g * P:(g + 1) * P].rearrange(
                        'c h w -> c (h w)'))
                nc.vector.tensor_reduce(xst[g][:, b:b + 1], st[:],
                                        axis=AX.X, op=AL.min)
                nc.vector.tensor_reduce(xst[g][:, BL + b:BL + b + 1], st[:],
                                        axis=AX.X, op=AL.max)

            # --- AG1: per-core sums of per-sample min/max (2 floats) ---
            negx = [perm.tile([P, BL], f32, name=f'negx{g}')
                    for g in range(G)]
            for g in range(G):
                nc.vector.tensor_scalar(negx[g][:], xst[g][:, 0:BL], -1.0,
                                        None, op0=AL.mult)
            qxl = qparams([negx[g][:] for g in range(G)],
                          [xst[g][:, BL:2 * BL] for g in range(G)],
                          BL, BL, 'xl')  # local per-core mean (count=BL)
            # payload: per-core [sum_negmin, sum_max] (recover via *BL)
            pay1 = perm.tile([1, 2], f32, name='pay1')
            nsum_l = pt('nsums_x')
            nc.vector.tensor_scalar(nsum_l[:], qxl['mn'][:], -BL, None,
                                    op0=AL.mult)
            xsum_l = pt('xsums_x')
            nc.vector.tensor_scalar(xsum_l[:], qxl['mx'][:], BL, None,
                                    op0=AL.mult)
            nc.vector.tensor_copy(pay1[:, 0:1], nsum_l[0:1, :])
            nc.vector.tensor_copy(pay1[:, 1:2], xsum_l[0:1, :])
            ag1_in = dpool.tile([2], f32, name='ag1_in')
            ag1_out = dpool.tile([NCORES * 2], f32, name='ag1_out')
            nc.sync.dma_start(ag1_in[None, :], pay1[:])
            nc.gpsimd.collective_compute(
                'AllGather', AL.bypass, replica_groups=rg,
                ins=[ag1_in[:].opt()], outs=[ag1_out[:].opt()])
            agb1 = perm.tile([1, NCORES * 2], f32, name='agb1')
            nc.sync.dma_start(agb1[:], ag1_out[None, :])
            agb1b = perm.tile([P, NCORES * 2], f32, name='agb1b')
            nc.gpsimd.partition_broadcast(agb1b[:], agb1[:])
            v1 = agb1b.rearrange('p (c s) -> p s c', s=2)
            mnx = pt('mn_x')
            nc.vector.tensor_reduce(mnx[:], v1[:, 0], axis=AX.X, op=AL.add)
            nc.vector.tensor_scalar(mnx[:], mnx[:], -1.0 / B_FULL, None,
                                    op0=AL.mult)
            mxx = pt('mx_x')
            nc.vector.tensor_reduce(mxx[:], v1[:, 1], axis=AX.X, op=AL.add)
            nc.vector.tensor_scalar(mxx[:], mxx[:], 1.0 / B_FULL, None,
                                    op0=AL.mult)
            qx = _qp_from_mnmx(mnx, mxx, 'x')

            # runtime depthwise scales: sws = s_w*s_x ; lam = mn_w*s_x
            sws = pt('sws')
            nc.vector.tensor_mul(sws[:], qx['s'][:], dwscb[:, 0:1])
            lam = pt('lam')
            nc.vector.tensor_mul(lam[:], qx['s'][:], dwscb[:, 1:2])
            # strip/corner consts scaled by -mn_x / +mn_x
            strC = perm.tile([P, G, 4], f32, name='strC')
            nc.vector.tensor_scalar(strC[:], strips[:], qx['negmn'][:, 0:1],
                                    None, op0=AL.mult)
            corC = perm.tile([P, G, 4], f32, name='corC')
            nc.vector.tensor_scalar(corC[:], corners[:], qx['mn'][:, 0:1],
                                    None, op0=AL.mult)
            const1 = perm.tile([P, G], f32, name='const1')
            nc.vector.scalar_tensor_tensor(
                const1[:], wsum_t[:], qx['mn'][:, 0:1], qdb_t[:],
                op0=AL.mult, op1=AL.add)

            # =========================================================
            # Phase B+C per tile: quantize x -> k (fp16); kh = horizontal
            # 3-sum (gpsimd); depthwise = 9 integer taps (fp16, psum A) +
            # 3 ident taps on kh (fp16, psum B); evict A (scalar act,
            # scale sws, bias const1) then merge B via V stt (*lam, +)
            # with accum_out; strips/corners; stats.
            # =========================================================
            h1 = {}
            CENTER = 4
            TAPS = [CENTER] + [t for t in range(9) if t != CENTER]
            for ti, (g, b) in enumerate(tiles):
                st = xr[(g, b)]
                nc.scalar.activation(st[:], st[:], AF.Relu,
                                     bias=qx['bias'][:, 0:1],
                                     scale=qx['inv_s'][:, 0:1])
                nc.vector.tensor_scalar(st[:], st[:], QMAX, MAGIC,
                                        op0=AL.min, op1=AL.add)
                kbt = kbp.tile([P, IMG], fp16, name=f'kb{g}_{b}', tag='kb')
                nc.vector.tensor_scalar(kbt[:], st[:], MAGIC, None,
                                        op0=AL.subtract)
                kbv = kbt.rearrange('p (h w) -> p h w', w=HH)

                h1t = big.tile([P, IMG], f32, name=f'h1_{g}_{b}',
                               tag='ring')
                h1[(g, b)] = h1t
                for half, subs in ((0, (0, 1, 2, 3)), (1, (4, 5, 6))):
                    psA = pspA.tile([P, 2048], f32, name=f'cvA{ti}_{half}',
                                    tag='psA')
                    psB = pspB.tile([P, 2048], f32, name=f'cvB{ti}_{half}',
                                    tag='psB')
                    for si, s in enumerate(subs):
                        r0 = 8 * s
                        ovA = psA[:, si * 512:si * 512 + 448].rearrange(
                            'p (r c) -> p r c', c=HH)
                        ovB = psB[:, si * 512:si * 512 + 448].rearrange(
                            'p (r c) -> p r c', c=HH)
                        for k, tap in enumerate(TAPS):
                            di, dj = tap // 3, tap % 3
                            irlo = max(0, r0 + di - 1)
                            irhi = min(HH, r0 + di + 7)
                            orlo = irlo - (r0 + di - 1)
                            nrows = irhi - irlo
                            oc0, ic0 = (1, 0) if dj == 0 else (
                                (0, 1) if dj == 2 else (0, 0))
                            ncols = 55 if dj != 1 else 56
                            lhs = diagq[:, (g * 9 + tap) * P:
                                        (g * 9 + tap + 1) * P]
                            nc.tensor.matmul(
                                ovA[:, orlo:orlo + nrows, oc0:oc0 + ncols],
                                lhs,
                                kbv[:, irlo:irhi, ic0:ic0 + ncols],
                                start=(k == 0), stop=(k == 8),
                                skip_group_check=True)
                        for k, di in enumerate((1, 0, 2)):
                            irlo = max(0, r0 + di - 1)
                            irhi = min(HH, r0 + di + 7)
                            orlo = irlo - (r0 + di - 1)
                            nrows = irhi - irlo
                            nc.tensor.matmul(
                                ovB[:, orlo:orlo + nrows, :], identh[:],
                                kbv[:, irlo:irhi, :],
                                start=(k == 0), stop=(k == 2),
                                skip_group_check=True)
                    nsub = len(subs)
                    # evict kv = V3(k) (ints <=765, exact fp16), then
                    # psB := H3(kv) (column-clipped ident taps)
                    kvt = kvp.tile([P, 1792], fp16, name=f'kv{ti}_{half}',
                                   tag='kv')
                    nc.scalar.activation(
                        kvt[:, 0:nsub * 448],
                        psB.rearrange('p (s x) -> p s x', s=4)[
                            :, 0:nsub, 0:448],
                        AF.Identity, bias=0.0, scale=1.0)
                    for si in range(nsub):
                        ovB2 = psB[:, si * 512:si * 512 + 448].rearrange(
                            'p (r c) -> p r c', c=HH)
                        kvv = kvt[:, si * 448:si * 448 + 448].rearrange(
                            'p (r c) -> p r c', c=HH)
                        for k, dj in enumerate((1, 0, 2)):
                            oc0, ic0 = (1, 0) if dj == 0 else (
                                (0, 1) if dj == 2 else (0, 0))
                            ncols = 55 if dj != 1 else 56
                            nc.tensor.matmul(
                                ovB2[:, :, oc0:oc0 + ncols], identh[:],
                                kvv[:, :, ic0:ic0 + ncols],
                                start=(k == 0), stop=(k == 2),
                                skip_group_check=True)
                    ivA = psA.rearrange('p (s x) -> p s x', s=4)[
                        :, 0:nsub, 0:448]
                    ivB = psB.rearrange('p (s x) -> p s x', s=4)[
                        :, 0:nsub, 0:448]
                    hseg = h1t[:, half * 1792:half * 1792 + nsub * 448]
                    nc.scalar.activation(hseg, ivA, AF.Identity,
                                         bias=const1[:, g:g + 1],
                                         scale=sws[:, 0:1])
                    nc.vector.scalar_tensor_tensor(
                        hseg, ivB, lam[:, 0:1], hseg,
                        op0=AL.mult, op1=AL.add,
                        accum_out=acc2[ti][:, half:half + 1])
                # strip + corner corrections on gpsimd
                h1v = h1t.rearrange('p (h w) -> p h w', w=HH)
                nc.gpsimd.tensor_scalar(h1t[:, 0:56], h1t[:, 0:56],
                                        strC[:, g, 0:1], None, op0=AL.add)
                nc.gpsimd.tensor_scalar(h1t[:, 3080:3136], h1t[:, 3080:3136],
                                        strC[:, g, 1:2], None, op0=AL.add)
                nc.gpsimd.tensor_scalar(h1v[:, :, 0:1], h1v[:, :, 0:1],
                                        strC[:, g, 2:3], None, op0=AL.add)
                nc.gpsimd.tensor_scalar(h1v[:, :, 55:56], h1v[:, :, 55:56],
                                        strC[:, g, 3:4], None, op0=AL.add)
                for ci, idx in enumerate((0, 55, 3080, 3135)):
                    nc.gpsimd.tensor_scalar(h1t[:, idx:idx + 1],
                                            h1t[:, idx:idx + 1],
                                            corC[:, g, ci:ci + 1], None,
                                            op0=AL.add)
                # stats (V only; gpsimd busy with kh in this phase)
                nc.vector.tensor_reduce(h1st[g][:, b:b + 1],
                                        h1t[:], axis=AX.X, op=AL.min)
                nc.vector.tensor_reduce(h1st[g][:, BL + b:BL + b + 1],
                                        h1t[:], axis=AX.X, op=AL.max)
                nc.vector.tensor_tensor(h1st[g][:, 2 * BL + b:2 * BL + b + 1],
                                        acc2[ti][:, 0:1], acc2[ti][:, 1:2],
                                        op=AL.add)

            # =========================================================
            # AG2: per-(c,b) h1 min/max/sum
            # =========================================================
            ag2_in = dpool.tile([G * P * 3 * BL], f32, name='ag2_in')
            ag2_out = dpool.tile([NCORES * G * P * 3 * BL], f32,
                                 name='ag2_out')
            v2i = ag2_in.rearrange('(g c f) -> g c f', g=G, c=P)
            for g in range(G):
                nc.sync.dma_start(v2i[g], h1st[g][:])
            nc.gpsimd.collective_compute(
                'AllGather', AL.bypass, replica_groups=rg,
                ins=[ag2_in[:].opt()], outs=[ag2_out[:].opt()])
            v2o = ag2_out.rearrange(
                '(core g c s b) -> g c s core b',
                core=NCORES, g=G, c=P, s=3, b=BL)
            for g in range(G):
                nc.sync.dma_start(Ag[g][:], v2o[g])

            # ---- stats post-processing (mirrors for h1 and h3) ----
            negm = [perm.tile([P, NCORES * BL], f32, name=f'negm{g}')
                    for g in range(G)]

            def stage_qparams(tag):
                for g in range(G):
                    nc.vector.tensor_scalar(
                        negm[g][:], Ag[g][:, 0].rearrange('p c b -> p (c b)'),
                        -1.0, None, op0=AL.mult)
                return qparams(
                    [negm[g][:] for g in range(G)],
                    [Ag[g][:, 1].rearrange('p c b -> p (c b)')
                     for g in range(G)],
                    NCORES * BL, B_FULL, tag)

            def chunk_stats():
                for g in range(G):
                    vv = Ag[g][:, 0].rearrange('p c (j k) -> p c j k', j=2)
                    nc.vector.tensor_reduce(
                        cstat[g][:, 0].rearrange('p (c j) -> p c j', c=NCORES),
                        vv, axis=AX.X, op=AL.min)
                    vv = Ag[g][:, 1].rearrange('p c (j k) -> p c j k', j=2)
                    nc.vector.tensor_reduce(
                        cstat[g][:, 1].rearrange('p (c j) -> p c j', c=NCORES),
                        vv, axis=AX.X, op=AL.max)

            # qchain on a small tile (in place): raw -> k ints
            def qchain_small(ap, q):
                nc.scalar.activation(ap, ap, AF.Relu, bias=q['bias'][:, 0:1],
                                     scale=q['inv_s'][:, 0:1])
                nc.vector.tensor_scalar(ap, ap, QMAX, MAGIC,
                                        op0=AL.min, op1=AL.add)
                nc.vector.tensor_scalar(ap, ap, MAGIC, None, op0=AL.subtract)

            # RangeBN scale: chunk stats -> quantized per-channel scale
            def rangebn_scale(q, tag):
                chunk_stats()
                scpk = perm.tile([P, G], f32, name=f'scpk_{tag}')
                for g in range(G):
                    c = cstat[g].rearrange('p s f -> p (s f)')
                    qchain_small(c[:, :], q)
                    mm = perm.tile([P, 2], f32, name=f'mm_{tag}{g}')
                    nc.vector.tensor_reduce(mm[:], cstat[g][:],
                                            axis=AX.X, op=AL.add)
                    nc.vector.tensor_scalar(mm[:], mm[:], 1.0 / NCHUNKS,
                                            q['s'][:, 0:1],
                                            op0=AL.mult, op1=AL.mult)
                    nc.vector.tensor_scalar(mm[:], mm[:], q['mn'][:, 0:1],
                                            None, op0=AL.add)
                    d = perm.tile([P, 1], f32, name=f'dmm_{tag}{g}')
                    nc.vector.tensor_sub(d[:], mm[:, 1:2], mm[:, 0:1])
                    nc.vector.tensor_scalar(d[:], d[:], SCALE_FIX, EPS,
                                            op0=AL.mult, op1=AL.add)
                    nc.vector.reciprocal(scpk[:, g:g + 1], d[:])
                # quantize scale over 256 channels (partition min/max)
                nsc = perm.tile([P, G], f32, name=f'nsc_{tag}')
                nc.vector.tensor_scalar(nsc[:], scpk[:], -1.0, None,
                                        op0=AL.mult)
                qs = qparams([nsc[:, g:g + 1] for g in range(G)],
                             [scpk[:, g:g + 1] for g in range(G)],
                             1, 1, f'sc_{tag}')
                qchain_small(scpk[:, :], qs)
                nc.vector.tensor_scalar(scpk[:], scpk[:], qs['s'][:, 0:1],
                                        None, op0=AL.mult)
                nc.vector.tensor_scalar(scpk[:], scpk[:], qs['mn'][:, 0:1],
                                        None, op0=AL.add)
                return scpk

            # BN coefficient block: returns cA, cB  (h2 = relu(cA*k + cB))
            def bn_coeffs(q, bnw_t, bnb_t, sum_adjust, tag):
                qscale = rangebn_scale(q, tag)
                A = perm.tile([P, G], f32, name=f'A_{tag}')
                nc.vector.tensor_mul(A[:], qscale[:], bnw_t[:])
                cA = perm.tile([P, G], f32, name=f'cA_{tag}')
                nc.vector.tensor_scalar(cA[:], A[:], q['s'][:, 0:1], None,
                                        op0=AL.mult)
                # mean = (sum_raw + adjust)/N ; cB = (mn - mean)*A + bnb
                sumh = perm.tile([P, G], f32, name=f'sumh_{tag}')
                for g in range(G):
                    nc.vector.tensor_reduce(
                        sumh[:, g:g + 1],
                        Ag[g][:, 2].rearrange('p c b -> p (c b)'),
                        axis=AX.X, op=AL.add)
                if sum_adjust is not None:
                    nc.vector.tensor_tensor(sumh[:], sumh[:], sum_adjust[:],
                                            op=AL.add)
                mean = perm.tile([P, G], f32, name=f'mean_{tag}')
                nc.vector.tensor_scalar(mean[:], sumh[:], 1.0 / N_TOT, None,
                                        op0=AL.mult)
                cB = perm.tile([P, G], f32, name=f'cB_{tag}')
                nc.vector.tensor_scalar(cB[:], mean[:], -1.0,
                                        q['mn'][:, 0:1],
                                        op0=AL.mult, op1=AL.add)
                nc.vector.tensor_mul(cB[:], cB[:], A[:])
                nc.vector.tensor_add(cB[:], cB[:], bnb_t[:])
                return cA, cB

            q1 = stage_qparams('h1')
            # sum adjustment: 32 * (-mn_x) * KS  per channel
            sadj = perm.tile([P, G], f32, name='sadj')
            n32 = pt('n32mnx')
            nc.vector.tensor_scalar(n32[:], qx['negmn'][:], float(B_FULL),
                                    None, op0=AL.mult)
            nc.vector.tensor_scalar(sadj[:], ks_t[:], n32[:, 0:1], None,
                                    op0=AL.mult)
            cA1, cB1 = bn_coeffs(q1, qbn1w_t, bn1b_t, sadj, 'bn1')

            # analytic qm(h2) bounds from Ag extremes (monotone, cA1>=0)
            for g in range(G):
                flat = Ag[g][:, 0:2].rearrange('p s c b -> p (s c b)')
                qchain_small(flat[:, :], q1)
                for s in range(2):
                    pl = Ag[g][:, s].rearrange('p c b -> p (c b)')
                    nc.scalar.activation(pl, pl, AF.Relu,
                                         bias=cB1[:, g:g + 1],
                                         scale=cA1[:, g:g + 1])
            q2 = stage_qparams('h2')
            # E coefficients: a2 = cA1/s2 ; b2r = 255 - (cB1-mn2)/s2
            a2 = perm.tile([P, G], f32, name='a2')
            nc.vector.tensor_scalar(a2[:], cA1[:], q2['inv_s'][:, 0:1], None,
                                    op0=AL.mult)
            b2r = perm.tile([P, G], f32, name='b2r')
            nc.vector.tensor_scalar(b2r[:], cB1[:], q2['mn'][:, 0:1],
                                    q2['inv_s'][:, 0:1],
                                    op0=AL.subtract, op1=AL.mult)
            nc.vector.tensor_scalar(b2r[:], b2r[:], -1.0, QMAX,
                                    op0=AL.mult, op1=AL.add)
            # scaled pointwise weights (split bf16 hi+lo) + const3
            pwTs = perm.tile([P, G * 256], f32, name='pwTs')
            nc.vector.tensor_scalar(pwTs[:], pwT[:], q2['s'][:, 0:1], None,
                                    op0=AL.mult)
            pwHI = perm.tile([P, G * 256], bf16, name='pwHI')
            nc.vector.tensor_copy(pwHI[:], pwTs[:])
            pwLO = perm.tile([P, G * 256], bf16, name='pwLO')
            nc.vector.tensor_sub(pwLO[:], pwTs[:], pwHI[:])
            const3 = perm.tile([P, G], f32, name='const3')
            nc.vector.tensor_scalar(const3[:], pwsum_t[:], q2['mn'][:, 0:1],
                                    None, op0=AL.mult)

            # =========================================================
            # Phase D/E per tile: h1 -> k1 -> k2 (bf16);
            # Phase F per batch: pointwise conv (split-bf16) + evict + stats
            # =========================================================
            h3 = {}
            k2 = {}
            for bb in range(BL):
                for g in range(G):
                    h1t = h1[(g, bb)]
                    u = scr.tile([P, IMG], f32, name=f'u_{g}_{bb}',
                                 tag='scr')
                    nc.scalar.activation(u[:], h1t[:], AF.Relu,
                                         bias=q1['bias'][:, 0:1],
                                         scale=q1['inv_s'][:, 0:1])
                    nc.vector.tensor_scalar(u[:], u[:], QMAX, MAGIC,
                                            op0=AL.min, op1=AL.add)
                    nc.vector.tensor_scalar(u[:], u[:], MAGIC,
                                            a2[:, g:g + 1],
                                            op0=AL.subtract, op1=AL.mult)
                    # clip via 255-z double-relu, then round -> k2 (bf16)
                    nc.scalar.activation(u[:], u[:], AF.Relu,
                                         bias=b2r[:, g:g + 1], scale=-1.0)
                    nc.scalar.activation(u[:], u[:], AF.Relu,
                                         bias=qmax_bc[:, 0:1], scale=-1.0)
                    k2t = k2p.tile([P, IMG], bf16, name=f'k2_{g}_{bb}',
                                   tag='k2')
                    k2[(g, bb)] = k2t
                    nc.vector.tensor_scalar(k2t[:], u[:], MAGIC, MAGIC,
                                            op0=AL.add, op1=AL.subtract)
                # pointwise conv for batch bb
                for cg in range(G):
                    ti = 2 * bb + cg
                    h3t = big.tile([P, IMG], f32, name=f'h3_{cg}_{bb}',
                                   tag='ring')
                    h3[(cg, bb)] = h3t
                    for half, nsub in ((0, 4), (1, 3)):
                        pool = pspA if half == 0 else pspB
                        pst = pool.tile([P, 2048], f32,
                                        name=f'pw{cg}_{bb}_{half}',
                                        tag='psA' if half == 0 else 'psB')
                        mi = 0
                        for kg in range(G):
                            for wt in (pwHI, pwLO):
                                lhs = wt[:, kg * 256 + cg * P:
                                         kg * 256 + (cg + 1) * P]
                                for si in range(nsub):
                                    c0 = (half * 4 + si) * 448
                                    nc.tensor.matmul(
                                        pst[:, si * 512:si * 512 + 448],
                                        lhs,
                                        k2[(kg, bb)][:, c0:c0 + 448],
                                        start=(mi == 0), stop=(mi == 3),
                                        skip_group_check=True)
                                mi += 1
                        iv2 = pst.rearrange('p (s x) -> p s x', s=4)[
                            :, 0:nsub, 0:448]
                        nc.scalar.activation(
                            h3t[:, half * 1792:half * 1792 + nsub * 448],
                            iv2, AF.Identity, bias=const3[:, cg:cg + 1],
                            scale=1.0, accum_out=acc2[ti][:, half:half + 1])
                    nc.vector.tensor_reduce(h3st[cg][:, bb:bb + 1],
                                            h3t[:], axis=AX.X, op=AL.min)
                    nc.vector.tensor_reduce(h3st[cg][:, BL + bb:BL + bb + 1],
                                            h3t[:], axis=AX.X, op=AL.max)
                    nc.vector.tensor_tensor(
                        h3st[cg][:, 2 * BL + bb:2 * BL + bb + 1],
                        acc2[ti][:, 0:1], acc2[ti][:, 1:2], op=AL.add)

            # =========================================================
            # AG5 + BN2 chain
            # =========================================================
            ag5_in = dpool.tile([G * P * 3 * BL], f32, name='ag5_in')
            ag5_out = dpool.tile([NCORES * G * P * 3 * BL], f32,
                                 name='ag5_out')
            v5i = ag5_in.rearrange('(g c f) -> g c f', g=G, c=P)
            for g in range(G):
                nc.sync.dma_start(v5i[g], h3st[g][:])
            nc.gpsimd.collective_compute(
                'AllGather', AL.bypass, replica_groups=rg,
                ins=[ag5_in[:].opt()], outs=[ag5_out[:].opt()])
            v5o = ag5_out.rearrange(
                '(core g c s b) -> g c s core b',
                core=NCORES, g=G, c=P, s=3, b=BL)
            for g in range(G):
                nc.sync.dma_start(Ag[g][:], v5o[g])

            q3 = stage_qparams('h3')
            cA3, cB3 = bn_coeffs(q3, qbn2w_t, bn2b_t, None, 'bn2')

            # =========================================================
            # Phase G/H per tile: h3 -> k3 (V chain) -> out (S act) -> DMA
            # =========================================================
            for (g, b) in tiles:
                h3t = h3[(g, b)]
                u = scr.tile([P, IMG], f32, name=f'u3_{g}_{b}', tag='scr')
                nc.vector.tensor_scalar(u[:], h3t[:], q3['inv_s'][:, 0:1],
                                        q3['bias'][:, 0:1],
                                        op0=AL.mult, op1=AL.add)
                nc.vector.tensor_scalar(u[:], u[:], 0.0, QMAX,
                                        op0=AL.max, op1=AL.min)
                nc.vector.tensor_scalar(u[:], u[:], MAGIC, MAGIC,
                                        op0=AL.add, op1=AL.subtract)
                nc.scalar.activation(h3t[:], u[:], AF.Relu,
                                     bias=cB3[:, g:g + 1],
                                     scale=cA3[:, g:g + 1])
                nc.sync.dma_start(
                    out_d[b, g * P:(g + 1) * P].rearrange(
                        'c h w -> c (h w)'), h3t[:])

    nc.compile()
    return nc


def _host_consts(dw_w, dw_b, bn1_w, bn1_b, pw_w, bn2_w, bn2_b):
    qint, qdw, s_w, mn_w = _host_quant_codes(dw_w)
    qint = qint.reshape(256, 9)
    qdw = qdw.reshape(256, 3, 3)
    qdb = _host_quant(dw_b)
    qpw = _host_quant(pw_w).reshape(256, 256)
    qbn1w = _host_quant(bn1_w)
    qbn2w = _host_quant(bn2_w)
    wsum = qdw.sum(axis=(1, 2), dtype=np.float32)
    wrow0 = qdw[:, 0, :].sum(axis=1, dtype=np.float32)
    wrow2 = qdw[:, 2, :].sum(axis=1, dtype=np.float32)
    wcol0 = qdw[:, :, 0].sum(axis=1, dtype=np.float32)
    wcol2 = qdw[:, :, 2].sum(axis=1, dtype=np.float32)
    strips = np.stack([wrow0, wrow2, wcol0, wcol2], axis=1)  # [256,4]
    corners = np.stack([qdw[:, 0, 0], qdw[:, 0, 2],
                        qdw[:, 2, 0], qdw[:, 2, 2]], axis=1)
    ksum = (HH * (wrow0 + wrow2 + wcol0 + wcol2)
            - corners.sum(axis=1)).astype(np.float32)
    pwsum = qpw.sum(axis=1, dtype=np.float32)
    # lhsT layout: pwT[kg, cin, (coutg*128 + cout)] = qpw[cout_full, kg*128+cin]
    pwT = np.ascontiguousarray(qpw.T.reshape(G, P, 256)).astype(np.float32)
    consts = {
        'ident': np.eye(P, dtype=np.float32),
        'qint': np.ascontiguousarray(qint.reshape(G, P, 9)),
        'dwsc': np.array([s_w, mn_w], dtype=np.float32),
        'wsum': wsum.reshape(G, P).copy(),
        'strips': np.ascontiguousarray(strips.reshape(G, P, 4)),
        'corners': np.ascontiguousarray(corners.reshape(G, P, 4)),
        'ksum': ksum.reshape(G, P).copy(),
        'qdb': qdb.reshape(G, P).copy(),
        'qbn1w': qbn1w.reshape(G, P).copy(),
        'bn1b': np.asarray(bn1_b, np.float32).reshape(G, P).copy(),
        'qbn2w': qbn2w.reshape(G, P).copy(),
        'bn2b': np.asarray(bn2_b, np.float32).reshape(G, P).copy(),
        'pwsum': pwsum.reshape(G, P).copy(),
        'pwT': pwT,
    }
    return consts


def make_in_maps(x, dw_w, dw_b, bn1_w, bn1_b, pw_w, bn2_w, bn2_b):
    x = np.asarray(x, np.float32)
    consts = _host_consts(dw_w, dw_b, bn1_w, bn1_b, pw_w, bn2_w, bn2_b)
    in_maps = []
    for c in range(NCORES):
        m = dict(consts)
        m['x'] = np.ascontiguousarray(x[c * BL:(c + 1) * BL])
        in_maps.append(m)
    return in_maps


def get_program(limit=7):
    if limit not in _PROGRAM_CACHE:
        _PROGRAM_CACHE[limit] = build_program(limit)
    return _PROGRAM_CACHE[limit]


def kernel(**inputs):
    from concourse.bass_utils import run_bass_kernel_spmd
    nc = get_program()
    in_maps = make_in_maps(**inputs)
    res = run_bass_kernel_spmd(nc, in_maps, core_ids=list(range(NCORES)))
    out = np.concatenate([res.results[i]['out'] for i in range(NCORES)],
                         axis=0)
    return out.astype(np.float32)


# revision 13
# speedup vs baseline: 1.0125x; 1.0125x over previous
"""Trainium2 Bass kernel for nn_DepthwiseSeparableFusedConv2d (v3).

Self-contained: takes FULL inputs (x [32,256,56,56] + weights), returns FULL
output [32,256,56,56].  Data-parallel over batch across 8 NeuronCores with 4
batches/core; channels on partitions in 2 groups of 128.

v3 design (v1 baseline ~968us, v2 ~472us but rel-err 1.87e-2):
 - EXACT integer depthwise conv: qdw*s_x = (s_w*s_x)*q + (mn_w*s_x)*1 with
   q the integer quant codes.  conv(k, q) runs in fp16 (ints <=255 exact,
   products exact in fp32 PSUM, fast weight load); the mn_w term is a 3x3
   box-sum = 3 vertical ident-matmul taps over a horizontally pre-summed
   fp16 tile kh (ints <=765 exact in fp16), computed on gpsimd.  The two
   PSUM groups are merged by a vector scalar_tensor_tensor which also
   harvests the per-(c,b) h1 sums via accum_out.  This removes the fp32r
   ~12-bit weight rounding that cost rel-err in v2.
 - pointwise conv as split-bf16 (hi+lo) 2-pass accumulation: k2 exact in
   bf16, weight splitting gives ~fp32 precision at bf16 speed.
 - x loaded contiguously straight into the ring ([p,3136] 12.5KB lines),
   quantized in place, k codes written to compact fp16 tiles (no padding;
   column-clipped taps + zero pads with analytic strip/corner corrections)
 - no DRAM spill of h3: 8-buffer SBUF ring (x raw -> h1 -> h3)
 - 3 collectives instead of 6 (sums ride with min/max; BN means from raw
   h sums); gpsimd partition_all_reduce for cross-channel stats
 - min/max stat reduces in phases A and F use a gpsimd tensor-tensor
   half-splitting prepass to take load off the vector engine
 - 2 warmup collectives absorb cc-stream init under the input DMA
"""

import math
import numpy as np

# ---------------------------------------------------------------- constants
P = 128
G = 2                 # channel groups (256 = 2*128)
B_FULL = 32
BL = 4                # batches per core
NCORES = 8
HH = 56
IMG = HH * HH         # 3136
MAGIC = 12582912.0    # 1.5 * 2**23  (fp32 round-to-nearest-even trick)
QMAX = 255.0
N_TOT = B_FULL * IMG  # 100352
NCHUNKS = 16
EPS = 1e-5
_N_CHUNK_EL = B_FULL * IMG // NCHUNKS
SCALE_FIX = float((0.5 * 0.35) * (1 + (math.pi * math.log(4)) ** 0.5)
                  / ((2 * math.log(_N_CHUNK_EL)) ** 0.5))

_PROGRAM_CACHE = {}


def _host_quant_codes(w):
    w = np.asarray(w, np.float32)
    mn = w.min()
    mx = w.max()
    scale = np.maximum(((mx - mn) / np.float32(QMAX)).astype(np.float32),
                       np.float32(1e-8))
    t = np.clip((w - mn) / scale, np.float32(0.0),
                np.float32(QMAX)).astype(np.float32)
    q = np.round(t).astype(np.float32)
    return q, (q * scale + mn).astype(np.float32), float(scale), float(mn)


def _host_quant(w):
    return _host_quant_codes(w)[1]


def build_program(limit=7):  # limit unused
    import concourse.bacc as bacc
    import concourse.bass_isa as bass_isa
    import concourse.mybir as mybir
    import concourse.tile as tile

    f32 = mybir.dt.float32
    bf16 = mybir.dt.bfloat16
    fp16 = mybir.dt.float16
    AL = mybir.AluOpType
    AF = mybir.ActivationFunctionType
    AX = mybir.AxisListType
    RED = bass_isa.ReduceOp

    nc = bacc.Bacc('TRN2', target_bir_lowering=False, debug=False,
                   num_devices=NCORES)

    # ------------------------------------------------ external tensors
    x_in = nc.dram_tensor('x', [BL, 256, HH, HH], f32, kind='ExternalInput')
    ident_in = nc.dram_tensor('ident', [P, P], f32, kind='ExternalInput')
    qint_in = nc.dram_tensor('qint', [G, P, 9], f32, kind='ExternalInput')
    dwsc_in = nc.dram_tensor('dwsc', [2], f32, kind='ExternalInput')
    wsum_in = nc.dram_tensor('wsum', [G, P], f32, kind='ExternalInput')
    strips_in = nc.dram_tensor('strips', [G, P, 4], f32,
                               kind='ExternalInput')
    corners_in = nc.dram_tensor('corners', [G, P, 4], f32,
                                kind='ExternalInput')
    ks_in = nc.dram_tensor('ksum', [G, P], f32, kind='ExternalInput')
    qdb_in = nc.dram_tensor('qdb', [G, P], f32, kind='ExternalInput')
    qbn1w_in = nc.dram_tensor('qbn1w', [G, P], f32, kind='ExternalInput')
    bn1b_in = nc.dram_tensor('bn1b', [G, P], f32, kind='ExternalInput')
    qbn2w_in = nc.dram_tensor('qbn2w', [G, P], f32, kind='ExternalInput')
    bn2b_in = nc.dram_tensor('bn2b', [G, P], f32, kind='ExternalInput')
    pwsum_in = nc.dram_tensor('pwsum', [G, P], f32, kind='ExternalInput')
    # pwT[kg, cin(128), (coutg, cout)] : lhsT layout, already transposed
    pwT_in = nc.dram_tensor('pwT', [G, P, 256], f32, kind='ExternalInput')
    out_d = nc.dram_tensor('out', [BL, 256, HH, HH], f32,
                           kind='ExternalOutput')

    rg = [list(range(NCORES))]
    tiles = [(g, b) for b in range(BL) for g in range(G)]  # b-major

    with tile.TileContext(nc) as tc:
        with (
            tc.tile_pool(name='perm', bufs=1) as perm,
            tc.tile_pool(name='big', bufs=8) as big,
            tc.tile_pool(name='kb', bufs=2) as kbp,
            tc.tile_pool(name='kv', bufs=2) as kvp,
            tc.tile_pool(name='k2', bufs=4) as k2p,
            tc.tile_pool(name='scr', bufs=2) as scr,
            tc.tile_pool(name='dram', bufs=1, space='DRAM') as dpool,
            tc.tile_pool(name='psA', bufs=2, space='PSUM') as pspA,
            tc.tile_pool(name='psB', bufs=2, space='PSUM') as pspB,
        ):
            # ------------------------------------------------ warmup AGs
            warm = perm.tile([1, 2], f32, name='warm')
            nc.vector.memset(warm[:], 0.0)
            for wi in range(2):
                agw_in = dpool.tile([2], f32, name=f'agw_in{wi}')
                agw_out = dpool.tile([NCORES * 2], f32, name=f'agw_out{wi}')
                nc.sync.dma_start(agw_in[None, :], warm[:])
                nc.gpsimd.collective_compute(
                    'AllGather', AL.bypass, replica_groups=rg,
                    ins=[agw_in[:].opt()], outs=[agw_out[:].opt()])

            # ------------------------------------------------ constants
            ident = perm.tile([P, P], f32, name='identsb')
            nc.sync.dma_start(ident[:], ident_in[:])
            qint = perm.tile([P, G * 9], f32, name='qintsb')
            nc.sync.dma_start(
                qint.rearrange('c (g t) -> c g t', g=G),
                qint_in.rearrange('g c t -> c g t'))
            dwsc = perm.tile([1, 2], f32, name='dwsc1')
            nc.sync.dma_start(dwsc[:], dwsc_in[None, :])
            dwscb = perm.tile([P, 2], f32, name='dwscb')
            nc.gpsimd.partition_broadcast(dwscb[:], dwsc[:])

            def load_gp(t_in, nm):
                t = perm.tile([P, G], f32, name=nm)
                nc.sync.dma_start(t[:], t_in.rearrange('g c -> c g'))
                return t
            wsum_t = load_gp(wsum_in, 'wsumsb')
            ks_t = load_gp(ks_in, 'kssb')
            qdb_t = load_gp(qdb_in, 'qdbsb')
            qbn1w_t = load_gp(qbn1w_in, 'qbn1wsb')
            bn1b_t = load_gp(bn1b_in, 'bn1bsb')
            qbn2w_t = load_gp(qbn2w_in, 'qbn2wsb')
            bn2b_t = load_gp(bn2b_in, 'bn2bsb')
            pwsum_t = load_gp(pwsum_in, 'pwsumsb')
            strips = perm.tile([P, G, 4], f32, name='stripssb')
            nc.sync.dma_start(strips[:], strips_in.rearrange('g c s -> c g s'))
            corners = perm.tile([P, G, 4], f32, name='cornerssb')
            nc.sync.dma_start(corners[:],
                              corners_in.rearrange('g c s -> c g s'))
            pwT = perm.tile([P, G * 256], f32, name='pwTsb')
            nc.sync.dma_start(pwT.rearrange('c (g m) -> c g m', g=G),
                              pwT_in[:].rearrange('g c m -> c g m'))

            # integer diag weights (fp16, exact) + fp16 identity
            diagq = perm.tile([P, G * 9 * P], fp16, name='diagq')
            for g in range(G):
                for t in range(9):
                    i = g * 9 + t
                    nc.vector.tensor_scalar(
                        diagq[:, i * P:(i + 1) * P], ident[:],
                        qint[:, i:i + 1], None, op0=AL.mult)
            identh = perm.tile([P, P], fp16, name='identh')
            nc.vector.tensor_scalar(identh[:], ident[:], 1.0, None,
                                    op0=AL.mult)

            # ------------------------------------------------ stat tiles
            # layout per group: [P, 12] = min(b0..3), max(b0..3), sum(b0..3)
            xst = [perm.tile([P, 2 * BL], f32, name=f'xst{g}')
                   for g in range(G)]
            h1st = [perm.tile([P, 3 * BL], f32, name=f'h1st{g}')
                    for g in range(G)]
            h3st = [perm.tile([P, 3 * BL], f32, name=f'h3st{g}')
                    for g in range(G)]
            Ag = [perm.tile([P, 3, NCORES, BL], f32, name=f'Ag{g}')
                  for g in range(G)]
            cstat = [perm.tile([P, 2, NCHUNKS], f32, name=f'cstat{g}')
                     for g in range(G)]
            acc2 = [perm.tile([P, 4], f32, name=f'acc2_{i}')
                    for i in range(len(tiles))]

            def pt(nm, w=1):
                return perm.tile([P, w], f32, name=nm)

            qmax_bc = pt('qmax_bc')
            nc.vector.memset(qmax_bc[:], QMAX)

            # helper: [P,1] per-sample quant params from neg-min/max planes
            def qparams(negs, maxs, F, count, tag):
                pn = []
                px = []
                for g in range(G):
                    a = perm.tile([P, F], f32, name=f'pn_{tag}{g}')
                    nc.gpsimd.partition_all_reduce(a[:], negs[g], channels=P,
                                                   reduce_op=RED.max)
                    pn.append(a)
                    b = perm.tile([P, F], f32, name=f'px_{tag}{g}')
                    nc.gpsimd.partition_all_reduce(b[:], maxs[g], channels=P,
                                                   reduce_op=RED.max)
                    px.append(b)
                nm = perm.tile([P, F], f32, name=f'nm_{tag}')
                nc.vector.tensor_tensor(nm[:], pn[0][:], pn[1][:], op=AL.max)
                mx = perm.tile([P, F], f32, name=f'mxp_{tag}')
                nc.vector.tensor_tensor(mx[:], px[0][:], px[1][:], op=AL.max)
                nsum = pt(f'nsum_{tag}')
                nc.vector.tensor_reduce(nsum[:], nm[:], axis=AX.X, op=AL.add)
                xsum = pt(f'xsum_{tag}')
                nc.vector.tensor_reduce(xsum[:], mx[:], axis=AX.X, op=AL.add)
                mn = pt(f'mn_{tag}')
                nc.vector.tensor_scalar(mn[:], nsum[:], -1.0 / count, None,
                                        op0=AL.mult)
                mxm = pt(f'mx_{tag}')
                nc.vector.tensor_scalar(mxm[:], xsum[:], 1.0 / count, None,
                                        op0=AL.mult)
                return _qp_from_mnmx(mn, mxm, tag)

            def _qp_from_mnmx(mn, mxm, tag):
                d = pt(f'd_{tag}')
                nc.vector.tensor_sub(d[:], mxm[:], mn[:])
                s = pt(f's_{tag}')
                nc.vector.tensor_scalar(s[:], d[:], 1.0 / QMAX, 1e-8,
                                        op0=AL.mult, op1=AL.max)
                inv_s = pt(f'invs_{tag}')
                nc.vector.reciprocal(inv_s[:], s[:])
                negmn = pt(f'negmn_{tag}')
                nc.vector.tensor_scalar(negmn[:], mn[:], -1.0, None,
                                        op0=AL.mult)
                bias = pt(f'bias_{tag}')
                nc.vector.tensor_mul(bias[:], negmn[:], inv_s[:])
                return {'mn': mn, 'mx': mxm, 's': s, 'inv_s': inv_s,
                        'negmn': negmn, 'bias': bias}

            # =========================================================
            # Phase A: load x (contiguous) straight into ring tiles,
            # per-(c,b) min/max (gpsimd half-split + V reduce)
            # =========================================================
            xr = {}
            for ti, (g, b) in enumerate(tiles):
                st = big.tile([P, IMG], f32, name=f'xr{g}_{b}', tag='ring')
                xr[(g, b)] = st
                nc.sync.dma_start(
                    st[:], x_in[b, g * P:(g + 1) * P].rearrange(
                        'c h w -> c (h w)'))
                nc.vector.tensor_reduce(xst[g][:, b:b + 1], st[:],
                                        axis=AX.X, op=AL.min)
                nc.vector.pool(xst[g][:, BL + b:BL + b + 1], st[:],
                               mybir.PoolFunctionType.max)

            # --- AG1: per-core sums of per-sample min/max (2 floats) ---
            negx = [perm.tile([P, BL], f32, name=f'negx{g}')
                    for g in range(G)]
            for g in range(G):
                nc.vector.tensor_scalar(negx[g][:], xst[g][:, 0:BL], -1.0,
                                        None, op0=AL.mult)
            qxl = qparams([negx[g][:] for g in range(G)],
                          [xst[g][:, BL:2 * BL] for g in range(G)],
                          BL, BL, 'xl')  # local per-core mean (count=BL)
            # payload: per-core [sum_negmin, sum_max] (recover via *BL)
            pay1 = perm.tile([1, 2], f32, name='pay1')
            nsum_l = pt('nsums_x')
            nc.vector.tensor_scalar(nsum_l[:], qxl['mn'][:], -BL, None,
                                    op0=AL.mult)
            xsum_l = pt('xsums_x')
            nc.vector.tensor_scalar(xsum_l[:], qxl['mx'][:], BL, None,
                                    op0=AL.mult)
            nc.vector.tensor_copy(pay1[:, 0:1], nsum_l[0:1, :])
            nc.vector.tensor_copy(pay1[:, 1:2], xsum_l[0:1, :])
            ag1_in = dpool.tile([2], f32, name='ag1_in')
            ag1_out = dpool.tile([NCORES * 2], f32, name='ag1_out')
            nc.sync.dma_start(ag1_in[None, :], pay1[:])
            nc.gpsimd.collective_compute(
                'AllGather', AL.bypass, replica_groups=rg,
                ins=[ag1_in[:].opt()], outs=[ag1_out[:].opt()])
            agb1 = perm.tile([1, NCORES * 2], f32, name='agb1')
            nc.sync.dma_start(agb1[:], ag1_out[None, :])
            agb1b = perm.tile([P, NCORES * 2], f32, name='agb1b')
            nc.gpsimd.partition_broadcast(agb1b[:], agb1[:])
            v1 = agb1b.rearrange('p (c s) -> p s c', s=2)
            mnx = pt('mn_x')
            nc.vector.tensor_reduce(mnx[:], v1[:, 0], axis=AX.X, op=AL.add)
            nc.vector.tensor_scalar(mnx[:], mnx[:], -1.0 / B_FULL, None,
                                    op0=AL.mult)
            mxx = pt('mx_x')
            nc.vector.tensor_reduce(mxx[:], v1[:, 1], axis=AX.X, op=AL.add)
            nc.vector.tensor_scalar(mxx[:], mxx[:], 1.0 / B_FULL, None,
                                    op0=AL.mult)
            qx = _qp_from_mnmx(mnx, mxx, 'x')

            # runtime depthwise scales: sws = s_w*s_x ; lam = mn_w*s_x
            sws = pt('sws')
            nc.vector.tensor_mul(sws[:], qx['s'][:], dwscb[:, 0:1])
            lam = pt('lam')
            nc.vector.tensor_mul(lam[:], qx['s'][:], dwscb[:, 1:2])
            # strip/corner consts scaled by -mn_x / +mn_x
            strC = perm.tile([P, G, 4], f32, name='strC')
            nc.vector.tensor_scalar(strC[:], strips[:], qx['negmn'][:, 0:1],
                                    None, op0=AL.mult)
            corC = perm.tile([P, G, 4], f32, name='corC')
            nc.vector.tensor_scalar(corC[:], corners[:], qx['mn'][:, 0:1],
                                    None, op0=AL.mult)
            const1 = perm.tile([P, G], f32, name='const1')
            nc.vector.scalar_tensor_tensor(
                const1[:], wsum_t[:], qx['mn'][:, 0:1], qdb_t[:],
                op0=AL.mult, op1=AL.add)

            # =========================================================
            # Phase B+C per tile: quantize x -> k (fp16); kh = horizontal
            # 3-sum (gpsimd); depthwise = 9 integer taps (fp16, psum A) +
            # 3 ident taps on kh (fp16, psum B); evict A (scalar act,
            # scale sws, bias const1) then merge B via V stt (*lam, +)
            # with accum_out; strips/corners; stats.
            # =========================================================
            h1 = {}
            CENTER = 4
            TAPS = [CENTER] + [t for t in range(9) if t != CENTER]
            for ti, (g, b) in enumerate(tiles):
                st = xr[(g, b)]
                nc.scalar.activation(st[:], st[:], AF.Relu,
                                     bias=qx['bias'][:, 0:1],
                                     scale=qx['inv_s'][:, 0:1])
                nc.vector.tensor_scalar(st[:], st[:], QMAX, MAGIC,
                                        op0=AL.min, op1=AL.add)
                kbt = kbp.tile([P, IMG], fp16, name=f'kb{g}_{b}', tag='kb')
                nc.vector.tensor_scalar(kbt[:], st[:], MAGIC, None,
                                        op0=AL.subtract)
                kbv = kbt.rearrange('p (h w) -> p h w', w=HH)

                h1t = big.tile([P, IMG], f32, name=f'h1_{g}_{b}',
                               tag='ring')
                h1[(g, b)] = h1t
                for qi, subs in enumerate(((0, 1), (2, 3), (4, 5), (6,))):
                    psA = pspA.tile([P, 1024], f32, name=f'cvA{ti}_{qi}',
                                    tag='pa')
                    psB = pspB.tile([P, 1024], f32, name=f'cvB{ti}_{qi}',
                                    tag='pb')
                    nsub = len(subs)
                    for si, s in enumerate(subs):
                        r0 = 8 * s
                        ovA = psA[:, si * 512:si * 512 + 448].rearrange(
                            'p (r c) -> p r c', c=HH)
                        ovB = psB[:, si * 512:si * 512 + 448].rearrange(
                            'p (r c) -> p r c', c=HH)
                        for k, tap in enumerate(TAPS):
                            di, dj = tap // 3, tap % 3
                            irlo = max(0, r0 + di - 1)
                            irhi = min(HH, r0 + di + 7)
                            orlo = irlo - (r0 + di - 1)
                            nrows = irhi - irlo
                            oc0, ic0 = (1, 0) if dj == 0 else (
                                (0, 1) if dj == 2 else (0, 0))
                            ncols = 55 if dj != 1 else 56
                            lhs = diagq[:, (g * 9 + tap) * P:
                                        (g * 9 + tap + 1) * P]
                            nc.tensor.matmul(
                                ovA[:, orlo:orlo + nrows, oc0:oc0 + ncols],
                                lhs,
                                kbv[:, irlo:irhi, ic0:ic0 + ncols],
                                start=(k == 0), stop=(k == 8),
                                skip_group_check=True)
                        for k, di in enumerate((1, 0, 2)):
                            irlo = max(0, r0 + di - 1)
                            irhi = min(HH, r0 + di + 7)
                            orlo = irlo - (r0 + di - 1)
                            nrows = irhi - irlo
                            nc.tensor.matmul(
                                ovB[:, orlo:orlo + nrows, :], identh[:],
                                kbv[:, irlo:irhi, :],
                                start=(k == 0), stop=(k == 2),
                                skip_group_check=True)
                    # evict kv = V3(k) (ints <=765, exact fp16), then
                    # psB := H3(kv) (column-clipped ident taps)
                    kvt = kvp.tile([P, 896], fp16, name=f'kv{ti}_{qi}',
                                   tag='kv')
                    nc.scalar.activation(
                        kvt[:, 0:nsub * 448],
                        psB.rearrange('p (s x) -> p s x', s=2)[
                            :, 0:nsub, 0:448],
                        AF.Identity, bias=0.0, scale=1.0)
                    for si in range(nsub):
                        ovB2 = psB[:, si * 512:si * 512 + 448].rearrange(
                            'p (r c) -> p r c', c=HH)
                        kvv = kvt[:, si * 448:si * 448 + 448].rearrange(
                            'p (r c) -> p r c', c=HH)
                        for k, dj in enumerate((1, 0, 2)):
                            oc0, ic0 = (1, 0) if dj == 0 else (
                                (0, 1) if dj == 2 else (0, 0))
                            ncols = 55 if dj != 1 else 56
                            nc.tensor.matmul(
                                ovB2[:, :, oc0:oc0 + ncols], identh[:],
                                kvv[:, :, ic0:ic0 + ncols],
                                start=(k == 0), stop=(k == 2),
                                skip_group_check=True)
                    ivA = psA.rearrange('p (s x) -> p s x', s=2)[
                        :, 0:nsub, 0:448]
                    ivB = psB.rearrange('p (s x) -> p s x', s=2)[
                        :, 0:nsub, 0:448]
                    qoff = 2 * 448 * qi
                    hseg = h1t[:, qoff:qoff + nsub * 448]
                    nc.scalar.activation(hseg, ivA, AF.Identity,
                                         bias=const1[:, g:g + 1],
                                         scale=sws[:, 0:1])
                    nc.vector.scalar_tensor_tensor(
                        hseg, ivB, lam[:, 0:1], hseg,
                        op0=AL.mult, op1=AL.add,
                        accum_out=acc2[ti][:, qi:qi + 1])
                # strip + corner corrections on gpsimd
                h1v = h1t.rearrange('p (h w) -> p h w', w=HH)
                nc.gpsimd.tensor_scalar(h1t[:, 0:56], h1t[:, 0:56],
                                        strC[:, g, 0:1], None, op0=AL.add)
                nc.gpsimd.tensor_scalar(h1t[:, 3080:3136], h1t[:, 3080:3136],
                                        strC[:, g, 1:2], None, op0=AL.add)
                nc.gpsimd.tensor_scalar(h1v[:, :, 0:1], h1v[:, :, 0:1],
                                        strC[:, g, 2:3], None, op0=AL.add)
                nc.gpsimd.tensor_scalar(h1v[:, :, 55:56], h1v[:, :, 55:56],
                                        strC[:, g, 3:4], None, op0=AL.add)
                for ci, idx in enumerate((0, 55, 3080, 3135)):
                    nc.gpsimd.tensor_scalar(h1t[:, idx:idx + 1],
                                            h1t[:, idx:idx + 1],
                                            corC[:, g, ci:ci + 1], None,
                                            op0=AL.add)
                # stats (V only; gpsimd busy with kh in this phase)
                nc.vector.tensor_reduce(h1st[g][:, b:b + 1],
                                        h1t[:], axis=AX.X, op=AL.min)
                nc.vector.tensor_reduce(h1st[g][:, BL + b:BL + b + 1],
                                        h1t[:], axis=AX.X, op=AL.max)
                nc.vector.tensor_reduce(h1st[g][:, 2 * BL + b:2 * BL + b + 1],
                                        acc2[ti][:], axis=AX.X, op=AL.add)

            # =========================================================
            # AG2: per-(c,b) h1 min/max/sum
            # =========================================================
            ag2_in = dpool.tile([G * P * 3 * BL], f32, name='ag2_in')
            ag2_out = dpool.tile([NCORES * G * P * 3 * BL], f32,
                                 name='ag2_out')
            v2i = ag2_in.rearrange('(g c f) -> g c f', g=G, c=P)
            for g in range(G):
                nc.sync.dma_start(v2i[g], h1st[g][:])
            nc.gpsimd.collective_compute(
                'AllGather', AL.bypass, replica_groups=rg,
                ins=[ag2_in[:].opt()], outs=[ag2_out[:].opt()])
            v2o = ag2_out.rearrange(
                '(core g c s b) -> g c s core b',
                core=NCORES, g=G, c=P, s=3, b=BL)
            for g in range(G):
                nc.sync.dma_start(Ag[g][:], v2o[g])

            # ---- stats post-processing (mirrors for h1 and h3) ----
            negm = [perm.tile([P, NCORES * BL], f32, name=f'negm{g}')
                    for g in range(G)]

            def stage_qparams(tag):
                for g in range(G):
                    nc.vector.tensor_scalar(
                        negm[g][:], Ag[g][:, 0].rearrange('p c b -> p (c b)'),
                        -1.0, None, op0=AL.mult)
                return qparams(
                    [negm[g][:] for g in range(G)],
                    [Ag[g][:, 1].rearrange('p c b -> p (c b)')
                     for g in range(G)],
                    NCORES * BL, B_FULL, tag)

            def chunk_stats():
                for g in range(G):
                    vv = Ag[g][:, 0].rearrange('p c (j k) -> p c j k', j=2)
                    nc.vector.tensor_reduce(
                        cstat[g][:, 0].rearrange('p (c j) -> p c j', c=NCORES),
                        vv, axis=AX.X, op=AL.min)
                    vv = Ag[g][:, 1].rearrange('p c (j k) -> p c j k', j=2)
                    nc.vector.tensor_reduce(
                        cstat[g][:, 1].rearrange('p (c j) -> p c j', c=NCORES),
                        vv, axis=AX.X, op=AL.max)

            # qchain on a small tile (in place): raw -> k ints
            def qchain_small(ap, q):
                nc.scalar.activation(ap, ap, AF.Relu, bias=q['bias'][:, 0:1],
                                     scale=q['inv_s'][:, 0:1])
                nc.vector.tensor_scalar(ap, ap, QMAX, MAGIC,
                                        op0=AL.min, op1=AL.add)
                nc.vector.tensor_scalar(ap, ap, MAGIC, None, op0=AL.subtract)

            # RangeBN scale: chunk stats -> quantized per-channel scale
            def rangebn_scale(q, tag):
                chunk_stats()
                scpk = perm.tile([P, G], f32, name=f'scpk_{tag}')
                for g in range(G):
                    c = cstat[g].rearrange('p s f -> p (s f)')
                    qchain_small(c[:, :], q)
                    mm = perm.tile([P, 2], f32, name=f'mm_{tag}{g}')
                    nc.vector.tensor_reduce(mm[:], cstat[g][:],
                                            axis=AX.X, op=AL.add)
                    nc.vector.tensor_scalar(mm[:], mm[:], 1.0 / NCHUNKS,
                                            q['s'][:, 0:1],
                                            op0=AL.mult, op1=AL.mult)
                    nc.vector.tensor_scalar(mm[:], mm[:], q['mn'][:, 0:1],
                                            None, op0=AL.add)
                    d = perm.tile([P, 1], f32, name=f'dmm_{tag}{g}')
                    nc.vector.tensor_sub(d[:], mm[:, 1:2], mm[:, 0:1])
                    nc.vector.tensor_scalar(d[:], d[:], SCALE_FIX, EPS,
                                            op0=AL.mult, op1=AL.add)
                    nc.vector.reciprocal(scpk[:, g:g + 1], d[:])
                # quantize scale over 256 channels (partition min/max)
                nsc = perm.tile([P, G], f32, name=f'nsc_{tag}')
                nc.vector.tensor_scalar(nsc[:], scpk[:], -1.0, None,
                                        op0=AL.mult)
                qs = qparams([nsc[:, g:g + 1] for g in range(G)],
                             [scpk[:, g:g + 1] for g in range(G)],
                             1, 1, f'sc_{tag}')
                qchain_small(scpk[:, :], qs)
                nc.vector.tensor_scalar(scpk[:], scpk[:], qs['s'][:, 0:1],
                                        None, op0=AL.mult)
                nc.vector.tensor_scalar(scpk[:], scpk[:], qs['mn'][:, 0:1],
                                        None, op0=AL.add)
                return scpk

            # BN coefficient block: returns cA, cB  (h2 = relu(cA*k + cB))
            def bn_coeffs(q, bnw_t, bnb_t, sum_adjust, tag):
                qscale = rangebn_scale(q, tag)
                A = perm.tile([P, G], f32, name=f'A_{tag}')
                nc.vector.tensor_mul(A[:], qscale[:], bnw_t[:])
                cA = perm.tile([P, G], f32, name=f'cA_{tag}')
                nc.vector.tensor_scalar(cA[:], A[:], q['s'][:, 0:1], None,
                                        op0=AL.mult)
                # mean = (sum_raw + adjust)/N ; cB = (mn - mean)*A + bnb
                sumh = perm.tile([P, G], f32, name=f'sumh_{tag}')
                for g in range(G):
                    nc.vector.tensor_reduce(
                        sumh[:, g:g + 1],
                        Ag[g][:, 2].rearrange('p c b -> p (c b)'),
                        axis=AX.X, op=AL.add)
                if sum_adjust is not None:
                    nc.vector.tensor_tensor(sumh[:], sumh[:], sum_adjust[:],
                                            op=AL.add)
                mean = perm.tile([P, G], f32, name=f'mean_{tag}')
                nc.vector.tensor_scalar(mean[:], sumh[:], 1.0 / N_TOT, None,
                                        op0=AL.mult)
                cB = perm.tile([P, G], f32, name=f'cB_{tag}')
                nc.vector.tensor_scalar(cB[:], mean[:], -1.0,
                                        q['mn'][:, 0:1],
                                        op0=AL.mult, op1=AL.add)
                nc.vector.tensor_mul(cB[:], cB[:], A[:])
                nc.vector.tensor_add(cB[:], cB[:], bnb_t[:])
                return cA, cB

            q1 = stage_qparams('h1')
            # sum adjustment: 32 * (-mn_x) * KS  per channel
            sadj = perm.tile([P, G], f32, name='sadj')
            n32 = pt('n32mnx')
            nc.vector.tensor_scalar(n32[:], qx['negmn'][:], float(B_FULL),
                                    None, op0=AL.mult)
            nc.vector.tensor_scalar(sadj[:], ks_t[:], n32[:, 0:1], None,
                                    op0=AL.mult)
            cA1, cB1 = bn_coeffs(q1, qbn1w_t, bn1b_t, sadj, 'bn1')

            # analytic qm(h2) bounds from Ag extremes (monotone, cA1>=0)
            for g in range(G):
                flat = Ag[g][:, 0:2].rearrange('p s c b -> p (s c b)')
                qchain_small(flat[:, :], q1)
                for s in range(2):
                    pl = Ag[g][:, s].rearrange('p c b -> p (c b)')
                    nc.scalar.activation(pl, pl, AF.Relu,
                                         bias=cB1[:, g:g + 1],
                                         scale=cA1[:, g:g + 1])
            q2 = stage_qparams('h2')
            # E coefficients: a2 = cA1/s2 ; b2r = 255 - (cB1-mn2)/s2
            a2 = perm.tile([P, G], f32, name='a2')
            nc.vector.tensor_scalar(a2[:], cA1[:], q2['inv_s'][:, 0:1], None,
                                    op0=AL.mult)
            b2r = perm.tile([P, G], f32, name='b2r')
            nc.vector.tensor_scalar(b2r[:], cB1[:], q2['mn'][:, 0:1],
                                    q2['inv_s'][:, 0:1],
                                    op0=AL.subtract, op1=AL.mult)
            nc.vector.tensor_scalar(b2r[:], b2r[:], -1.0, QMAX,
                                    op0=AL.mult, op1=AL.add)
            # scaled pointwise weights (split bf16 hi+lo) + const3
            pwTs = perm.tile([P, G * 256], f32, name='pwTs')
            nc.vector.tensor_scalar(pwTs[:], pwT[:], q2['s'][:, 0:1], None,
                                    op0=AL.mult)
            pwHI = perm.tile([P, G * 256], bf16, name='pwHI')
            nc.vector.tensor_copy(pwHI[:], pwTs[:])
            pwLO = perm.tile([P, G * 256], bf16, name='pwLO')
            nc.vector.tensor_sub(pwLO[:], pwTs[:], pwHI[:])
            const3 = perm.tile([P, G], f32, name='const3')
            nc.vector.tensor_scalar(const3[:], pwsum_t[:], q2['mn'][:, 0:1],
                                    None, op0=AL.mult)

            # =========================================================
            # Phase D/E per tile: h1 -> k1 -> k2 (bf16);
            # Phase F per batch: pointwise conv (split-bf16) + evict + stats
            # =========================================================
            h3 = {}
            k2 = {}
            for bb in range(BL):
                for g in range(G):
                    h1t = h1[(g, bb)]
                    u = scr.tile([P, IMG], f32, name=f'u_{g}_{bb}',
                                 tag='scr')
                    nc.scalar.activation(u[:], h1t[:], AF.Relu,
                                         bias=q1['bias'][:, 0:1],
                                         scale=q1['inv_s'][:, 0:1])
                    nc.vector.tensor_scalar(u[:], u[:], QMAX, MAGIC,
                                            op0=AL.min, op1=AL.add)
                    nc.vector.tensor_scalar(u[:], u[:], MAGIC,
                                            a2[:, g:g + 1],
                                            op0=AL.subtract, op1=AL.mult)
                    # clip via 255-z double-relu, then round -> k2 (bf16)
                    nc.scalar.activation(u[:], u[:], AF.Relu,
                                         bias=b2r[:, g:g + 1], scale=-1.0)
                    nc.scalar.activation(u[:], u[:], AF.Relu,
                                         bias=qmax_bc[:, 0:1], scale=-1.0)
                    k2t = k2p.tile([P, IMG], bf16, name=f'k2_{g}_{bb}',
                                   tag='k2')
                    k2[(g, bb)] = k2t
                    nc.vector.tensor_scalar(k2t[:], u[:], MAGIC, MAGIC,
                                            op0=AL.add, op1=AL.subtract)
                # pointwise conv for batch bb
                for cg in range(G):
                    ti = 2 * bb + cg
                    h3t = big.tile([P, IMG], f32, name=f'h3_{cg}_{bb}',
                                   tag='ring')
                    h3[(cg, bb)] = h3t
                    for qi, subs in enumerate(((0, 1), (2, 3), (4, 5),
                                               (6,))):
                        pool = pspA if qi % 2 == 0 else pspB
                        pst = pool.tile([P, 1024], f32,
                                        name=f'pw{cg}_{bb}_{qi}',
                                        tag='pa' if qi % 2 == 0 else 'pb')
                        nsub = len(subs)
                        mi = 0
                        for kg in range(G):
                            for wt in (pwHI, pwLO):
                                lhs = wt[:, kg * 256 + cg * P:
                                         kg * 256 + (cg + 1) * P]
                                for si, s in enumerate(subs):
                                    c0 = s * 448
                                    nc.tensor.matmul(
                                        pst[:, si * 512:si * 512 + 448],
                                        lhs,
                                        k2[(kg, bb)][:, c0:c0 + 448],
                                        start=(mi == 0), stop=(mi == 3),
                                        skip_group_check=True)
                                mi += 1
                        iv2 = pst.rearrange('p (s x) -> p s x', s=2)[
                            :, 0:nsub, 0:448]
                        qoff = 2 * 448 * qi
                        nc.scalar.activation(
                            h3t[:, qoff:qoff + nsub * 448],
                            iv2, AF.Identity, bias=const3[:, cg:cg + 1],
                            scale=1.0, accum_out=acc2[ti][:, qi:qi + 1])
                    nc.vector.tensor_reduce(h3st[cg][:, bb:bb + 1],
                                            h3t[:], axis=AX.X, op=AL.min)
                    nc.vector.pool(h3st[cg][:, BL + bb:BL + bb + 1],
                                   h3t[:], mybir.PoolFunctionType.max)
                    nc.vector.tensor_reduce(
                        h3st[cg][:, 2 * BL + bb:2 * BL + bb + 1],
                        acc2[ti][:], axis=AX.X, op=AL.add)

            # =========================================================
            # AG5 + BN2 chain
            # =========================================================
            ag5_in = dpool.tile([G * P * 3 * BL], f32, name='ag5_in')
            ag5_out = dpool.tile([NCORES * G * P * 3 * BL], f32,
                                 name='ag5_out')
            v5i = ag5_in.rearrange('(g c f) -> g c f', g=G, c=P)
            for g in range(G):
                nc.sync.dma_start(v5i[g], h3st[g][:])
            nc.gpsimd.collective_compute(
                'AllGather', AL.bypass, replica_groups=rg,
                ins=[ag5_in[:].opt()], outs=[ag5_out[:].opt()])
            v5o = ag5_out.rearrange(
                '(core g c s b) -> g c s core b',
                core=NCORES, g=G, c=P, s=3, b=BL)
            for g in range(G):
                nc.sync.dma_start(Ag[g][:], v5o[g])

            q3 = stage_qparams('h3')
            cA3, cB3 = bn_coeffs(q3, qbn2w_t, bn2b_t, None, 'bn2')

            # =========================================================
            # Phase G/H per tile: h3 -> k3 (V chain) -> out (S act) -> DMA
            # =========================================================
            for (g, b) in tiles:
                h3t = h3[(g, b)]
                u = scr.tile([P, IMG], f32, name=f'u3_{g}_{b}', tag='scr')
                nc.scalar.activation(u[:], h3t[:], AF.Relu,
                                     bias=q3['bias'][:, 0:1],
                                     scale=q3['inv_s'][:, 0:1])
                nc.vector.tensor_scalar(u[:], u[:], QMAX, MAGIC,
                                        op0=AL.min, op1=AL.add)
                nc.vector.tensor_scalar(u[:], u[:], MAGIC, None,
                                        op0=AL.subtract)
                nc.scalar.activation(h3t[:], u[:], AF.Relu,
                                     bias=cB3[:, g:g + 1],
                                     scale=cA3[:, g:g + 1])
                nc.sync.dma_start(
                    out_d[b, g * P:(g + 1) * P].rearrange(
                        'c h w -> c (h w)'), h3t[:])

    nc.compile()
    return nc


def _host_consts(dw_w, dw_b, bn1_w, bn1_b, pw_w, bn2_w, bn2_b):
    qint, qdw, s_w, mn_w = _host_quant_codes(dw_w)
    qint = qint.reshape(256, 9)
    qdw = qdw.reshape(256, 3, 3)
    qdb = _host_quant(dw_b)
    qpw = _host_quant(pw_w).reshape(256, 256)
    qbn1w = _host_quant(bn1_w)
    qbn2w = _host_quant(bn2_w)
    wsum = qdw.sum(axis=(1, 2), dtype=np.float32)
    wrow0 = qdw[:, 0, :].sum(axis=1, dtype=np.float32)
    wrow2 = qdw[:, 2, :].sum(axis=1, dtype=np.float32)
    wcol0 = qdw[:, :, 0].sum(axis=1, dtype=np.float32)
    wcol2 = qdw[:, :, 2].sum(axis=1, dtype=np.float32)
    strips = np.stack([wrow0, wrow2, wcol0, wcol2], axis=1)  # [256,4]
    corners = np.stack([qdw[:, 0, 0], qdw[:, 0, 2],
                        qdw[:, 2, 0], qdw[:, 2, 2]], axis=1)
    ksum = (HH * (wrow0 + wrow2 + wcol0 + wcol2)
            - corners.sum(axis=1)).astype(np.float32)
    pwsum = qpw.sum(axis=1, dtype=np.float32)
    # lhsT layout: pwT[kg, cin, (coutg*128 + cout)] = qpw[cout_full, kg*128+cin]
    pwT = np.ascontiguousarray(qpw.T.reshape(G, P, 256)).astype(np.float32)
    consts = {
        'ident': np.eye(P, dtype=np.float32),
        'qint': np.ascontiguousarray(qint.reshape(G, P, 9)),
        'dwsc': np.array([s_w, mn_w], dtype=np.float32),
        'wsum': wsum.reshape(G, P).copy(),
        'strips': np.ascontiguousarray(strips.reshape(G, P, 4)),
        'corners': np.ascontiguousarray(corners.reshape(G, P, 4)),
        'ksum': ksum.reshape(G, P).copy(),
        'qdb': qdb.reshape(G, P).copy(),
        'qbn1w': qbn1w.reshape(G, P).copy(),
        'bn1b': np.asarray(bn1_b, np.float32).reshape(G, P).copy(),
        'qbn2w': qbn2w.reshape(G, P).copy(),
        'bn2b': np.asarray(bn2_b, np.float32).reshape(G, P).copy(),
        'pwsum': pwsum.reshape(G, P).copy(),
        'pwT': pwT,
    }
    return consts


def make_in_maps(x, dw_w, dw_b, bn1_w, bn1_b, pw_w, bn2_w, bn2_b):
    x = np.asarray(x, np.float32)
    consts = _host_consts(dw_w, dw_b, bn1_w, bn1_b, pw_w, bn2_w, bn2_b)
    in_maps = []
    for c in range(NCORES):
        m = dict(consts)
        m['x'] = np.ascontiguousarray(x[c * BL:(c + 1) * BL])
        in_maps.append(m)
    return in_maps


def get_program(limit=7):
    if limit not in _PROGRAM_CACHE:
        _PROGRAM_CACHE[limit] = build_program(limit)
    return _PROGRAM_CACHE[limit]


def kernel(**inputs):
    from concourse.bass_utils import run_bass_kernel_spmd
    nc = get_program()
    in_maps = make_in_maps(**inputs)
    res = run_bass_kernel_spmd(nc, in_maps, core_ids=list(range(NCORES)))
    out = np.concatenate([res.results[i]['out'] for i in range(NCORES)],
                         axis=0)
    return out.astype(np.float32)
